# revision 8
# baseline (speedup 1.0000x reference)
"""BatchPC whitening kernel for 8 Trainium2 NeuronCores.

Pipeline (data-parallel over the batch dim, 262144 rows/core):
  1. Gram launch: split x = hi + lo (bf16 pair, exact to 2^-16) on
     ACT/DVE, then accumulate [hi|lo].T @ [hi|lo] per 128-row group on
     the TensorEngine in bf16 (products are exact in fp32 PSUM; the four
     64x64 blocks sum to the f32-exact Gram). Also emits hi (bf16 x) as
     a scratch output for pass 2.
  2. Host: combine the 8 partial Grams in f64, momentum-update the
     running covariance, eigh (64x64, f64), build the whitening map Q.
  3. Apply launch: hardware DMA-transpose loads of the bf16 scratch
     (2-byte dtype makes the xbar path legal) feed bf16 matmuls against
     a [Q^T;Q^T] block-diagonal stack — no TensorE transposes at all.

x is loaded as [128, 512] tiles holding 8 consecutive rows per partition
(2KB contiguous DMA descriptors per partition, full HBM bandwidth).
"""

import ml_dtypes
import numpy as np

import concourse.bacc as bacc
import concourse.mybir as mybir
import concourse.tile as tile
from concourse.bass import ds, ts
from concourse.bass_utils import run_bass_kernel_spmd

NCORES = 8
N = 2097152
DIN = 64
DOUT = 32
MOMENTUM = 0.1
NI = N // NCORES          # 262144 rows per core
ROWS_PER_TILE = 1024      # one [128, 512] f32 SBUF tile
F32 = mybir.dt.float32
BF16 = mybir.dt.bfloat16

_NC_CACHE = {}
LAST_EXEC_NS = []  # exec_time_ns per launch when BASS_TRACE is on


def _gram_program(ni):
    nt = ni // ROWS_PER_TILE
    nc = bacc.Bacc(None)
    x = nc.declare_dram_parameter("x", [ni, DIN], F32, isOutput=False)
    g = nc.declare_dram_parameter("gram", [128, 128], F32, isOutput=True)
    xbf = nc.declare_dram_parameter("xbf", [ni, DIN], BF16, isOutput=True)
    # row (n*1024 + p*8 + t) -> tile n, partition p, free (t*64 + d)
    xv = x.rearrange("(n p t) d -> n p (t d)", p=128, t=8)
    bv = xbf.rearrange("(n p t) d -> n p (t d)", p=128, t=8)
    with tile.TileContext(nc) as tc:
        with (
            tc.tile_pool(name="xin", bufs=4) as xp,
            tc.tile_pool(name="hi", bufs=4) as hp,
            tc.tile_pool(name="z", bufs=4) as zp,
            tc.tile_pool(name="acc", bufs=1, space="PSUM") as pp,
            tc.tile_pool(name="gout", bufs=1) as gp,
        ):
            acc = pp.tile([128, 128], F32)
            for i in range(nt):
                xt = xp.tile([128, 512], F32)
                nc.sync.dma_start(xt[:], xv[i])
                hi = hp.tile([128, 512], BF16)
                z = zp.tile([128, 1024], BF16)     # (t, hl, d): [hi_t|lo_t] pairs
                z4 = z[:].rearrange("p (t hl d) -> p t hl d", t=8, hl=2)
                nc.scalar.copy(hi[:], xt[:])       # cast f32 -> bf16
                nc.vector.tensor_sub(z4[:, :, 1, :], xt[:], hi[:])  # lo = x - hi
                nc.gpsimd.tensor_copy(z4[:, :, 0, :], hi[:])
                nc.sync.dma_start(bv[i], hi[:])
                for t in range(8):
                    # [hi_t|lo_t].T @ [hi_t|lo_t]: 4 Gram cross-blocks
                    nc.tensor.matmul(
                        acc[:],
                        z[:, ts(t, 128)],
                        z[:, ts(t, 128)],
                        start=(i == 0 and t == 0),
                        stop=(i == nt - 1 and t == 7),
                    )
            gs = gp.tile([128, 128], F32)
            nc.vector.tensor_copy(gs[:], acc[:])
            nc.sync.dma_start(g[:], gs[:])
    nc.compile()
    return nc


def _apply_program(ni):
    ngroups = ni // 2048
    nc = bacc.Bacc(None)
    xbf = nc.declare_dram_parameter("xbf", [ni, DIN], BF16, isOutput=False)
    q2 = nc.declare_dram_parameter("q2", [128, 2 * DOUT], BF16, isOutput=False)
    out = nc.declare_dram_parameter("out", [ni, DOUT], F32, isOutput=True)
    # 512-row chunks as [256 pairs, 128]: xbar-transpose puts even rows on
    # partitions 0:64, odd rows on 64:128 (block-diagonal-ready layout)
    sv = xbf.rearrange("(h rp two) d -> h rp (two d)", rp=256, two=2)
    # psum col (tt*256 + 64k + 32s + c) <-> row (g*2048 + tt*1024 + k*256 + m*2 + s)
    ov = out.rearrange("(g tt k m s) c -> g tt m k s c", tt=2, k=4, m=128, s=2)
    with tile.TileContext(nc) as tc:
        with (
            tc.tile_pool(name="const", bufs=1) as cp,
            tc.tile_pool(name="xT", bufs=4) as xtp,
            tc.tile_pool(name="oacc", bufs=2, space="PSUM") as oap,
            tc.tile_pool(name="osb", bufs=3) as osp,
        ):
            qt = cp.tile([128, 2 * DOUT], BF16)
            nc.sync.dma_start(qt[:], q2[:])
            for g in range(ngroups):
                oacc = oap.tile([128, 512], F32)
                for tt in range(2):
                    xT = xtp.tile([128, 512], BF16)
                    # each transposing DMA covers 512 rows -> [128, 256]
                    nc.sync.dma_start(xT[:, :256], sv[4 * g + 2 * tt], transpose=True)
                    nc.sync.dma_start(xT[:, 256:], sv[4 * g + 2 * tt + 1], transpose=True)
                    for k in range(4):
                        nc.tensor.matmul(
                            oacc[:, ds(256 * tt + 64 * k, 64)],
                            xT[:, ts(k, 128)],
                            qt[:],
                            start=True,
                            stop=True,
                        )
                osb = osp.tile([128, 512], F32)
                nc.vector.tensor_copy(osb[:], oacc[:])
                nc.sync.dma_start(ov[g, 0], osb[:, :256])
                nc.sync.dma_start(ov[g, 1], osb[:, 256:])
    nc.compile()
    return nc


def _run(nc, in_maps):
    res = run_bass_kernel_spmd(nc, in_maps, core_ids=list(range(NCORES)))
    if res.exec_time_ns is not None:
        LAST_EXEC_NS.append(res.exec_time_ns)
    return res.results


def _host_q(gram, rC, n):
    """f64 covariance update + eigh + whitening map; returns q2 stack (bf16)."""
    C = gram / n
    rC64 = rC.astype(np.float64)
    rC_new = rC64 + MOMENTUM * (C - rC64)
    es, ev = np.linalg.eigh(rC_new)
    es = es[::-1][:DOUT]
    ev = ev[:, ::-1][:, :DOUT].T              # [DOUT, DIN]
    pivot = np.linspace(0.0, 1.0, DIN).reshape(DIN, 1)
    ev = np.sign(ev @ pivot) * ev
    Q = ev / np.sqrt(es)[:, None]             # [DOUT, DIN]
    QT = np.ascontiguousarray(Q.T)            # [DIN, DOUT]
    q2 = np.zeros((128, 2 * DOUT), ml_dtypes.bfloat16)
    q2[:DIN, :DOUT] = QT.astype(ml_dtypes.bfloat16)
    q2[DIN:, DOUT:] = QT.astype(ml_dtypes.bfloat16)
    return q2


def kernel(x, rC):
    x = np.asarray(x)
    rC = np.asarray(rC)
    assert x.shape == (N, DIN) and rC.shape == (DIN, DIN)

    if "gram" not in _NC_CACHE:
        _NC_CACHE["gram"] = _gram_program(NI)
    if "apply" not in _NC_CACHE:
        _NC_CACHE["apply"] = _apply_program(NI)

    shards = [x[i * NI : (i + 1) * NI] for i in range(NCORES)]

    # ---- launch 1: partial Grams + bf16 scratch ----
    gres = _run(_NC_CACHE["gram"], [{"x": s} for s in shards])
    gram = np.zeros((DIN, DIN), np.float64)
    for i in range(NCORES):
        gb = gres[i]["gram"].astype(np.float64)
        gram += gb[:DIN, :DIN] + gb[:DIN, DIN:] + gb[DIN:, :DIN] + gb[DIN:, DIN:]

    q2 = _host_q(gram, rC, N)

    # ---- launch 2: out = x @ Q^T ----
    ares = _run(
        _NC_CACHE["apply"],
        [{"xbf": gres[i]["xbf"], "q2": q2} for i in range(NCORES)],
    )
    return np.concatenate([ares[i]["out"] for i in range(NCORES)], axis=0)


# revision 9
# speedup vs baseline: 1.9719x; 1.9719x over previous
"""BatchPC whitening kernel for 8 Trainium2 NeuronCores.

Pipeline (data-parallel over the batch dim, 262144 rows/core):
  1. Gram launch: each core accumulates its shard's partial x^T x on the
     TensorEngine in f32 (PSUM fp32 accumulation), pairing two 128-row
     groups per matmul so the two diagonal 64x64 blocks of the [128,128]
     accumulator sum to the shard Gram.
  2. Host: combine the 8 partial Grams in f64, momentum-update the
     running covariance, eigh (64x64, f64), build the whitening map Q.
  3. Apply launch: out = x @ Q^T. x tiles are transposed on the
     TensorEngine (f32 DMA transpose is unsupported); the PSUM->SBUF
     copy casts to bf16 so the apply matmuls run at bf16 rate against a
     [Q^T;Q^T] block-diagonal bf16 stack (adds ~0.2% benign error, far
     below the reference's own f32-eigh noise floor). Outputs are laid
     out so the store DMA is 1KB-contiguous per partition.

x is loaded as [128, 512] tiles holding 8 consecutive rows per partition
(2KB contiguous DMA descriptors per partition, full HBM bandwidth).
"""

import ml_dtypes
import numpy as np

import concourse.bacc as bacc
import concourse.mybir as mybir
import concourse.tile as tile
from concourse.bass import ds, ts
from concourse.bass_utils import run_bass_kernel_spmd
from concourse.masks import make_identity

NCORES = 8
N = 2097152
DIN = 64
DOUT = 32
MOMENTUM = 0.1
NI = N // NCORES          # 262144 rows per core
ROWS_PER_TILE = 1024      # one [128, 512] SBUF tile
F32 = mybir.dt.float32
BF16 = mybir.dt.bfloat16

_NC_CACHE = {}
LAST_EXEC_NS = []  # exec_time_ns per launch when BASS_TRACE is on


def _gram_program(ni):
    nt = ni // ROWS_PER_TILE
    nc = bacc.Bacc(None)
    x = nc.declare_dram_parameter("x", [ni, DIN], F32, isOutput=False)
    g = nc.declare_dram_parameter("gram", [128, 128], F32, isOutput=True)
    # row (n*1024 + p*8 + t) -> tile n, partition p, free (t*64 + d)
    xv = x.rearrange("(n p t) d -> n p (t d)", p=128, t=8)
    with tile.TileContext(nc) as tc:
        with (
            tc.tile_pool(name="xin", bufs=4) as xp,
            tc.tile_pool(name="acc", bufs=1, space="PSUM") as pp,
            tc.tile_pool(name="gout", bufs=1) as gp,
        ):
            acc = pp.tile([128, 128], F32)
            for i in range(nt):
                xt = xp.tile([128, 512], F32)
                nc.sync.dma_start(xt[:], xv[i])
                for j in range(4):
                    # [A|B].T @ [A|B]: diagonal 64x64 blocks are partial Grams
                    nc.tensor.matmul(
                        acc[:],
                        xt[:, ts(j, 128)],
                        xt[:, ts(j, 128)],
                        start=(i == 0 and j == 0),
                        stop=(i == nt - 1 and j == 3),
                    )
            gs = gp.tile([128, 128], F32)
            nc.vector.tensor_copy(gs[:], acc[:])
            nc.sync.dma_start(g[:], gs[:])
    nc.compile()
    return nc


def _apply_program(ni):
    nt = ni // ROWS_PER_TILE
    nc = bacc.Bacc(None)
    x = nc.declare_dram_parameter("x", [ni, DIN], F32, isOutput=False)
    q2 = nc.declare_dram_parameter("q2", [128, 2 * DOUT], BF16, isOutput=False)
    out = nc.declare_dram_parameter("out", [ni, DOUT], F32, isOutput=True)
    xv = x.rearrange("(n p t) d -> n p (t d)", p=128, t=8)
    # row (m*1024 + p*8 + j*2 + s) -> block m, partition p,
    # free (j*64 + s*32 + c): 8 rows x 32 cols = 1KB contiguous per partition
    ov = out.rearrange("(m p j s) c -> m p (j s c)", p=128, j=4, s=2)
    with tile.TileContext(nc) as tc:
        with (
            tc.tile_pool(name="const", bufs=1) as cp,
            tc.tile_pool(name="xin", bufs=4) as xp,
            tc.tile_pool(name="ptr", bufs=4, space="PSUM") as ptp,
            tc.tile_pool(name="xT", bufs=4) as xtp,
            tc.tile_pool(name="oacc", bufs=2, space="PSUM") as oap,
            tc.tile_pool(name="osb", bufs=3) as osp,
        ):
            ident = cp.tile([128, 128], F32)
            make_identity(nc, ident[:])
            qt = cp.tile([128, 2 * DOUT], BF16)
            nc.sync.dma_start(qt[:], q2[:])
            for gidx in range(nt // 2):
                oacc = oap.tile([128, 512], F32)
                for tt in range(2):
                    i = 2 * gidx + tt
                    xt = xp.tile([128, 512], F32)
                    nc.sync.dma_start(xt[:], xv[i])
                    for j in range(4):
                        pt = ptp.tile([128, 128], F32)
                        nc.tensor.transpose(pt[:], xt[:, ts(j, 128)], ident[:])
                        xT = xtp.tile([128, 128], BF16)
                        if j % 2 == 0:
                            nc.vector.tensor_copy(xT[:], pt[:])  # casts to bf16
                        else:
                            nc.scalar.copy(xT[:], pt[:])
                        nc.tensor.matmul(
                            oacc[:, ds(256 * tt + 64 * j, 64)],
                            xT[:],
                            qt[:],
                            start=True,
                            stop=True,
                        )
                osb = osp.tile([128, 512], F32)
                nc.vector.tensor_copy(osb[:], oacc[:])
                nc.sync.dma_start(ov[2 * gidx], osb[:, :256])
                nc.sync.dma_start(ov[2 * gidx + 1], osb[:, 256:])
    nc.compile()
    return nc


def _run(nc, in_maps):
    res = run_bass_kernel_spmd(nc, in_maps, core_ids=list(range(NCORES)))
    if res.exec_time_ns is not None:
        LAST_EXEC_NS.append(res.exec_time_ns)
    return res.results


def _host_q(gram, rC, n):
    """f64 covariance update + eigh + whitening map; returns q2 stack (bf16)."""
    C = gram / n
    rC64 = rC.astype(np.float64)
    rC_new = rC64 + MOMENTUM * (C - rC64)
    es, ev = np.linalg.eigh(rC_new)
    es = es[::-1][:DOUT]
    ev = ev[:, ::-1][:, :DOUT].T              # [DOUT, DIN]
    pivot = np.linspace(0.0, 1.0, DIN).reshape(DIN, 1)
    ev = np.sign(ev @ pivot) * ev
    Q = ev / np.sqrt(es)[:, None]             # [DOUT, DIN]
    QT = np.ascontiguousarray(Q.T)            # [DIN, DOUT]
    q2 = np.zeros((128, 2 * DOUT), ml_dtypes.bfloat16)
    q2[:DIN, :DOUT] = QT.astype(ml_dtypes.bfloat16)
    q2[DIN:, DOUT:] = QT.astype(ml_dtypes.bfloat16)
    return q2


def kernel(x, rC):
    x = np.asarray(x)
    rC = np.asarray(rC)
    assert x.shape == (N, DIN) and rC.shape == (DIN, DIN)

    if "gram" not in _NC_CACHE:
        _NC_CACHE["gram"] = _gram_program(NI)
    if "apply" not in _NC_CACHE:
        _NC_CACHE["apply"] = _apply_program(NI)

    shards = [x[i * NI : (i + 1) * NI] for i in range(NCORES)]

    # ---- launch 1: partial Grams ----
    gres = _run(_NC_CACHE["gram"], [{"x": s} for s in shards])
    gram = np.zeros((DIN, DIN), np.float64)
    for i in range(NCORES):
        gb = gres[i]["gram"].astype(np.float64)
        gram += gb[:DIN, :DIN] + gb[DIN:, DIN:]

    q2 = _host_q(gram, rC, N)

    # ---- launch 2: out = x @ Q^T ----
    ares = _run(_NC_CACHE["apply"], [{"x": s, "q2": q2} for s in shards])
    return np.concatenate([ares[i]["out"] for i in range(NCORES)], axis=0)


# revision 10
# speedup vs baseline: 2.1428x; 1.0867x over previous
"""BatchPC whitening kernel for 8 Trainium2 NeuronCores.

Pipeline (data-parallel over the batch dim, 262144 rows/core):
  1. Gram launch: each core accumulates its shard's partial x^T x on the
     TensorEngine in f32 (PSUM fp32 accumulation), pairing two 128-row
     groups per matmul so the two diagonal 64x64 blocks of the [128,128]
     accumulator sum to the shard Gram.
  2. Host: combine the 8 partial Grams in f64, momentum-update the
     running covariance, eigh (64x64, f64), build the whitening map Q.
  3. Apply launch: out = x @ Q^T. x tiles are transposed on the
     TensorEngine (f32 DMA transpose is unsupported); the PSUM->SBUF
     copy casts to bf16 so the apply matmuls run at bf16 rate against a
     [Q^T;Q^T] block-diagonal bf16 stack (adds ~0.2% benign error, far
     below the reference's own f32-eigh noise floor). Outputs are laid
     out so the store DMA is 1KB-contiguous per partition.

x is loaded as [128, 512] tiles holding 8 consecutive rows per partition
(2KB contiguous DMA descriptors per partition, full HBM bandwidth).
"""

import ml_dtypes
import numpy as np

import concourse.bacc as bacc
import concourse.mybir as mybir
import concourse.tile as tile
from concourse.bass import ds, ts
from concourse.bass_utils import run_bass_kernel_spmd
from concourse.masks import make_identity

NCORES = 8
N = 2097152
DIN = 64
DOUT = 32
MOMENTUM = 0.1
NI = N // NCORES          # 262144 rows per core
ROWS_PER_TILE = 1024      # one [128, 512] SBUF tile
F32 = mybir.dt.float32
BF16 = mybir.dt.bfloat16

_NC_CACHE = {}
LAST_EXEC_NS = []  # exec_time_ns per launch when BASS_TRACE is on


def _gram_program(ni):
    nt = ni // ROWS_PER_TILE
    nc = bacc.Bacc(None)
    x = nc.declare_dram_parameter("x", [ni, DIN], F32, isOutput=False)
    g = nc.declare_dram_parameter("gram", [128, 128], F32, isOutput=True)
    # row (n*1024 + p*8 + t) -> tile n, partition p, free (t*64 + d)
    xv = x.rearrange("(n p t) d -> n p (t d)", p=128, t=8)
    with tile.TileContext(nc) as tc:
        with (
            tc.tile_pool(name="xin", bufs=6) as xp,
            tc.tile_pool(name="acc", bufs=1, space="PSUM") as pp,
            tc.tile_pool(name="gout", bufs=1) as gp,
        ):
            acc = pp.tile([128, 128], F32)
            for i in range(nt):
                xt = xp.tile([128, 512], F32)
                nc.sync.dma_start(xt[:], xv[i])
                for j in range(4):
                    # [A|B].T @ [A|B]: diagonal 64x64 blocks are partial Grams
                    nc.tensor.matmul(
                        acc[:],
                        xt[:, ts(j, 128)],
                        xt[:, ts(j, 128)],
                        start=(i == 0 and j == 0),
                        stop=(i == nt - 1 and j == 3),
                    )
            gs = gp.tile([128, 128], F32)
            nc.vector.tensor_copy(gs[:], acc[:])
            nc.sync.dma_start(g[:], gs[:])
    nc.compile()
    return nc


def _apply_program(ni):
    nt = ni // ROWS_PER_TILE
    nc = bacc.Bacc(None)
    x = nc.declare_dram_parameter("x", [ni, DIN], F32, isOutput=False)
    q2 = nc.declare_dram_parameter("q2", [128, 2 * DOUT], BF16, isOutput=False)
    out = nc.declare_dram_parameter("out", [ni, DOUT], F32, isOutput=True)
    xv = x.rearrange("(n p t) d -> n p (t d)", p=128, t=8)
    # row (m*1024 + p*8 + j*2 + s) -> block m, partition p,
    # free (j*64 + s*32 + c): 8 rows x 32 cols = 1KB contiguous per partition
    ov = out.rearrange("(m p j s) c -> m p (j s c)", p=128, j=4, s=2)
    with tile.TileContext(nc) as tc:
        with (
            tc.tile_pool(name="const", bufs=1) as cp,
            tc.tile_pool(name="xin", bufs=6) as xp,
            tc.tile_pool(name="ptr", bufs=6, space="PSUM") as ptp,
            tc.tile_pool(name="xT", bufs=8) as xtp,
            tc.tile_pool(name="oacc", bufs=2, space="PSUM") as oap,
            tc.tile_pool(name="osb", bufs=4) as osp,
        ):
            ident = cp.tile([128, 128], F32)
            make_identity(nc, ident[:])
            qt = cp.tile([128, 2 * DOUT], BF16)
            nc.sync.dma_start(qt[:], q2[:])
            for gidx in range(nt // 2):
                oacc = oap.tile([128, 512], F32)
                for tt in range(2):
                    i = 2 * gidx + tt
                    xt = xp.tile([128, 512], F32)
                    nc.sync.dma_start(xt[:], xv[i])
                    for j in range(4):
                        pt = ptp.tile([128, 128], F32)
                        nc.tensor.transpose(pt[:], xt[:, ts(j, 128)], ident[:])
                        xT = xtp.tile([128, 128], BF16)
                        if j % 2 == 0:
                            nc.vector.tensor_copy(xT[:], pt[:])  # casts to bf16
                        else:
                            nc.scalar.copy(xT[:], pt[:])
                        nc.tensor.matmul(
                            oacc[:, ds(256 * tt + 64 * j, 64)],
                            xT[:],
                            qt[:],
                            start=True,
                            stop=True,
                        )
                osb = osp.tile([128, 512], F32)
                nc.vector.tensor_copy(osb[:], oacc[:])
                nc.sync.dma_start(ov[2 * gidx], osb[:, :256])
                nc.sync.dma_start(ov[2 * gidx + 1], osb[:, 256:])
    nc.compile()
    return nc


def _run(nc, in_maps):
    res = run_bass_kernel_spmd(nc, in_maps, core_ids=list(range(NCORES)))
    if res.exec_time_ns is not None:
        LAST_EXEC_NS.append(res.exec_time_ns)
    return res.results


def _host_q(gram, rC, n):
    """f64 covariance update + eigh + whitening map; returns q2 stack (bf16)."""
    C = gram / n
    rC64 = rC.astype(np.float64)
    rC_new = rC64 + MOMENTUM * (C - rC64)
    es, ev = np.linalg.eigh(rC_new)
    es = es[::-1][:DOUT]
    ev = ev[:, ::-1][:, :DOUT].T              # [DOUT, DIN]
    pivot = np.linspace(0.0, 1.0, DIN).reshape(DIN, 1)
    ev = np.sign(ev @ pivot) * ev
    Q = ev / np.sqrt(es)[:, None]             # [DOUT, DIN]
    QT = np.ascontiguousarray(Q.T)            # [DIN, DOUT]
    q2 = np.zeros((128, 2 * DOUT), ml_dtypes.bfloat16)
    q2[:DIN, :DOUT] = QT.astype(ml_dtypes.bfloat16)
    q2[DIN:, DOUT:] = QT.astype(ml_dtypes.bfloat16)
    return q2


def kernel(x, rC):
    x = np.asarray(x)
    rC = np.asarray(rC)
    assert x.shape == (N, DIN) and rC.shape == (DIN, DIN)

    if "gram" not in _NC_CACHE:
        _NC_CACHE["gram"] = _gram_program(NI)
    if "apply" not in _NC_CACHE:
        _NC_CACHE["apply"] = _apply_program(NI)

    shards = [x[i * NI : (i + 1) * NI] for i in range(NCORES)]

    # ---- launch 1: partial Grams ----
    gres = _run(_NC_CACHE["gram"], [{"x": s} for s in shards])
    gram = np.zeros((DIN, DIN), np.float64)
    for i in range(NCORES):
        gb = gres[i]["gram"].astype(np.float64)
        gram += gb[:DIN, :DIN] + gb[DIN:, DIN:]

    q2 = _host_q(gram, rC, N)

    # ---- launch 2: out = x @ Q^T ----
    ares = _run(_NC_CACHE["apply"], [{"x": s, "q2": q2} for s in shards])
    return np.concatenate([ares[i]["out"] for i in range(NCORES)], axis=0)
